# revision 17
# baseline (speedup 1.0000x reference)
"""BertSelfAttention (B=4, S=2048, D=768, H=12) on 8 Trainium2 NeuronCores.

Sharding: 8 cores = 4 batches x 2 head-groups (6 heads each). Per core,
for its (batch b, head-group g):

  Q^T = Wq_g^T @ x_b^T        [384, 2048] f32r  (d_local on partitions)
  K^T = Wk_g^T @ x_b^T        [384, 2048] f32r
  V   = x_b @ Wv_g            [2048, 384] -> bf16, ones-augmented per head
  per (head h, q-half):  flash-style kc pipeline
    S^T[k, q]  = sum_dh K^T[dh, k] Q^T[dh, q]        (PE f32r, psum [128,1024])
    P^T[k, q]  = exp(S^T/8 + mask[k])                (Act, fused -> bf16 SBUF)
    ctx[q, d|den] += P^T[k, q-chunk]^stat @ Vaug[k]  (PE bf16, q-major out,
                                                      ones col gives denom)
    out[q, d]  = ctx[q, 0:64] / ctx[q, 64]           (GpSimd normalize_recip
                                                      on the Pool engine)

The kernel is Act(exp)-bound: ~25.2M exp elements/core at 0.833 ns/elem,
~16.6 us of exp per (head, q-half) unit. Everything else hides in the
units' PE slack: scores run CTX_LAG kc ahead of ctx, the current rep's
mi=1/mi=2 projection chunks fill units 0-7, and the NEXT rep's input DMAs,
V projection, and mi=0 chunks run in units 6-11 (VB and the mi=0 Q^T/K^T
tiles are double-buffered across reps), so the steady-state per-rep period
approaches the pure-exp window.
"""

import numpy as np

import concourse.mybir as mybir
import concourse.tile as tile
from concourse import bacc
from concourse.bass_utils import run_bass_kernel_spmd

F32 = mybir.dt.float32
F32R = mybir.dt.float32r
BF16 = mybir.dt.bfloat16
U16 = mybir.dt.uint16
U32 = mybir.dt.uint32
ONE_BF16_BITS = 0x3F80

N_CORES = 8
B, S, D, H = 4, 2048, 768, 12
HL = 6           # heads per core
DH = 64          # head dim
DL = HL * DH     # 384: local output dim
DCH = D // 128   # 6 contraction chunks
MCH = DL // 128  # 3 partition chunks of Q^T/K^T (2 heads each)
SCH = S // 128   # 16 key chunks
VST = 66         # per-head stride in the ones-augmented V tile (64 d + 1 one)
QW = 1024        # q-columns per unit (psum/exp slice width)
NU = 2           # q-halves per head
CTX_LAG = 2      # kc distance between scores and ctx emission

_cached = {}


def build_program(reps=1, use_bias=False):
    """reps>1 repeats the whole computation in one NEFF - used only by
    test.py to amortize dispatch overhead when measuring HW exec time.

    use_bias=False omits the attention-mask bias port on the exp
    activation (worth ~0.4 us per exp instruction on HW); kernel()
    selects the bias build only when the mask is actually nonzero."""
    if ("nc", reps, use_bias) in _cached:
        return _cached[("nc", reps, use_bias)]
    nc = bacc.Bacc("TRN2", target_bir_lowering=False, debug=False, num_devices=1)
    xT = nc.dram_tensor("xT", [D, S], F32, kind="ExternalInput").ap()
    wq = nc.dram_tensor("wq", [D, DL], F32, kind="ExternalInput").ap()
    wk = nc.dram_tensor("wk", [D, DL], F32, kind="ExternalInput").ap()
    wv = nc.dram_tensor("wv", [D, DL], F32, kind="ExternalInput").ap()
    bq = nc.dram_tensor("bq", [128, MCH], F32, kind="ExternalInput").ap()
    bk = nc.dram_tensor("bk", [128, MCH], F32, kind="ExternalInput").ap()
    bv = nc.dram_tensor("bv", [1, DL], F32, kind="ExternalInput").ap()
    mask = nc.dram_tensor("mask", [128, SCH], F32, kind="ExternalInput").ap()
    out = nc.dram_tensor("out", [HL, S, DH], F32, kind="ExternalOutput").ap()

    EXP = mybir.ActivationFunctionType.Exp

    with tile.TileContext(nc) as tc, \
         tc.tile_pool(name="persist", bufs=1) as persist, \
         tc.tile_pool(name="load", bufs=1) as load, \
         tc.tile_pool(name="pj", bufs=2, space="PSUM") as pj, \
         tc.tile_pool(name="sps", bufs=2, space="PSUM") as sps, \
         tc.tile_pool(name="cps", bufs=1, space="PSUM") as cps, \
         tc.tile_pool(name="ptp", bufs=CTX_LAG + 3) as ptp, \
         tc.tile_pool(name="csp", bufs=2) as csp, \
         tc.tile_pool(name="osp", bufs=2) as osp:

        # ---- per-rep resource handles (double-buffered across reps where
        # the next rep's projections overlap this rep's attention) ----
        R = []
        for r in range(reps):
            par = r % 2 if reps > 1 else 0
            d = {}
            d["qt"] = [persist.tile([128, S], F32R, tag=f"qt{i}{par if i == 0 else 0}",
                                    name=f"qt{i}_r{r}") for i in range(MCH)]
            d["kt"] = [persist.tile([128, S], F32R, tag=f"kt{i}{par if i == 0 else 0}",
                                    name=f"kt{i}_r{r}") for i in range(MCH)]
            d["vb"] = [persist.tile([128, HL * VST], BF16, tag=f"vb{i}{par}",
                                    name=f"vb{i}_r{r}") for i in range(SCH)]
            d["xt"] = [load.tile([128, S], F32R, tag=f"xt{dc}", name=f"xt{dc}_r{r}")
                       for dc in range(DCH)]
            d["w"] = {nm: [load.tile([128, DL], F32R, tag=f"w{nm}{dc}",
                                     name=f"w{nm}{dc}_r{r}") for dc in range(DCH)]
                      for nm in ("q", "k", "v")}
            d["bq"] = load.tile([128, MCH], F32, tag="bq", name=f"bq_r{r}")
            d["bk"] = load.tile([128, MCH], F32, tag="bk", name=f"bk_r{r}")
            d["bv"] = load.tile([1, DL], F32, tag="bv", name=f"bv_r{r}")
            R.append(d)

        mask_sb = persist.tile([128, SCH], F32, tag="mask")
        nc.sync.dma_start(mask_sb[:], mask[:])
        for par in range(min(reps, 2)):
            for i in range(SCH):
                # ones column (col 64 of each head block): softmax denominator
                nc.vector.memset(
                    R[par]["vb"][i][:].bitcast(U16).rearrange(
                        "p (h j) -> p h j", j=VST)[:, :, 64:65],
                    ONE_BF16_BITS)
        ones_row = persist.tile([1, 128], F32, tag="ones_row")
        nc.vector.memset(ones_row[:], 1.0)

        def input_dmas(r):
            d = R[r]
            for dc in range(DCH):
                nc.sync.dma_start(d["w"]["q"][dc][:],
                                  wq[dc * 128:(dc + 1) * 128, :].bitcast(F32R))
            nc.sync.dma_start(d["bq"][:], bq[:])
            nc.sync.dma_start(d["bk"][:], bk[:])
            for dc in range(DCH):
                nc.sync.dma_start(d["xt"][dc][:],
                                  xT[dc * 128:(dc + 1) * 128, :].bitcast(F32R))
            for dc in range(DCH):
                nc.sync.dma_start(d["w"]["k"][dc][:],
                                  wk[dc * 128:(dc + 1) * 128, :].bitcast(F32R))
            for dc in range(DCH):
                nc.sync.dma_start(d["w"]["v"][dc][:],
                                  wv[dc * 128:(dc + 1) * 128, :].bitcast(F32R))
            nc.sync.dma_start(d["bv"][:], bv[:])

        def qk_chunk(r, wn, mi, q):
            """One [128, 512] column chunk of rep r's Q^T or K^T."""
            d = R[r]
            bt, dst = (d["bq"], d["qt"]) if wn == "q" else (d["bk"], d["kt"])
            ps = pj.tile([128, 512], F32, tag="qk", name="ps_qk")
            for dc in range(DCH):
                nc.tensor.matmul(
                    ps[:],
                    d["w"][wn][dc][:, mi * 128:(mi + 1) * 128],
                    d["xt"][dc][:, q * 512:(q + 1) * 512],
                    start=(dc == 0), stop=(dc == DCH - 1),
                )
            nc.vector.tensor_scalar_add(
                dst[mi][:, q * 512:(q + 1) * 512], ps[:], bt[:, mi:mi + 1])

        def v_chunk(r, sc):
            """One [128 seq, 384] chunk of rep r's V -> bf16 augmented VB."""
            d = R[r]
            pst = pj.tile([128, 512], F32, tag="qk", name="ps_v")
            ps = pst[:, 0:DL]
            for dc in range(DCH):
                nc.tensor.matmul(
                    ps,
                    d["xt"][dc][:, sc * 128:(sc + 1) * 128],
                    d["w"]["v"][dc][:],
                    start=(dc == 0), stop=False,
                )
            nc.tensor.matmul(ps, ones_row[:], d["bv"][:],
                             start=False, stop=True)
            nc.vector.tensor_copy(
                out=d["vb"][sc][:].rearrange("p (h j) -> p h j", j=VST)[:, :, 0:64],
                in_=ps.rearrange("p (h j) -> p h j", j=64),
            )

        # ---- insert schedule: jobs keyed by (rep, unit, kc) ----
        inserts = {}

        def add(r, u, kc, job):
            inserts.setdefault((r, u, kc), []).append(job)

        for r in range(reps):
            # this rep's mi=1 chunks in units 1-3 (deadline: unit 4 = h2)
            for j in range(8):
                u, kc = 1 + j // 3, (j % 3) * 5 + 1
                add(r, u, kc, lambda r=r, wn=("q", "k")[j % 2], q=j // 2:
                    qk_chunk(r, wn, 1, q))
            # this rep's mi=2 chunks in units 4-6 (deadline: unit 8 = h4;
            # all of this rep's xt reads end by unit 6 kc 8 so the next
            # rep's input DMAs can be emitted right after)
            for j in range(8):
                u, kc = 4 + j // 3, (j % 3) * 5 + 3
                add(r, u, kc, lambda r=r, wn=("q", "k")[j % 2], q=j // 2:
                    qk_chunk(r, wn, 2, q))
            if r == 0:
                # no previous rep: V and remaining mi=0 go into unit 0
                for sc in range(SCH):
                    add(0, 0, sc, lambda sc=sc: v_chunk(0, sc))
                add(0, 0, 1, lambda: qk_chunk(0, "k", 0, 1))
                add(0, 0, 5, lambda: qk_chunk(0, "k", 0, 2))
                add(0, 0, 9, lambda: qk_chunk(0, "k", 0, 3))
                add(0, 0, 11, lambda: qk_chunk(0, "q", 0, 2))
                add(0, 0, 13, lambda: qk_chunk(0, "q", 0, 3))
            if r + 1 < reps:
                # prefetch for the NEXT rep into this rep's tail units
                # (after this rep's last xt/w read at unit 6 kc 13)
                add(r, 6, 15, lambda r=r: input_dmas(r + 1))
                for sc in range(SCH):  # V: units 7-10, 4 chunks each
                    add(r, 7 + sc // 4, (sc % 4) * 4 + 2,
                        lambda r=r, sc=sc: v_chunk(r + 1, sc))
                for j in range(8):     # mi=0: units 9-11, 3/3/2 chunks
                    u, kc = 9 + j // 3, (j % 3) * 5 + 3
                    add(r, u, kc, lambda r=r, wn=("q", "k")[j % 2], q=j // 2:
                        qk_chunk(r + 1, wn, 0, q))

        # ---- rep 0 prologue ----
        input_dmas(0)
        # PE warm-up during the input-DMA window: junk matmuls ramp the
        # p-state so the first real chunks run at full speed.
        warm_in = persist.tile([1, 512], F32R, tag="warm")
        nc.vector.memset(warm_in[:].bitcast(U32), 0)
        warm_ps = pj.tile([128, 512], F32, tag="qk", name="warm_ps")
        for _ in range(14):
            nc.tensor.matmul(warm_ps[:], ones_row[:].bitcast(F32R),
                             warm_in[:], start=True, stop=True)
        # just enough of mi=0 for unit 0 to start (QT cols 0:1024 for the
        # moving operand, KT cols 0:512 covering kc 0-3)
        qk_chunk(0, "q", 0, 0)
        qk_chunk(0, "q", 0, 1)
        qk_chunk(0, "k", 0, 0)

        # ---- attention units ----
        for r in range(reps):
            d = R[r]
            for u in range(HL * NU):
                h, half = divmod(u, NU)
                mi, pr = h // 2, (h % 2) * 64
                q0 = half * QW
                c_ps = cps.tile([128, 1024], F32, tag="c", name="c_ps")
                pt_tiles = [None] * SCH

                def ctx(kc):
                    # PSUM accumulation groups are bank-granular (2 KB zero
                    # regions): start only on the first matmul touching each
                    # bank (pending-zero lazily zeroes the other slots on
                    # their first write), stop only on the last.
                    for qc in range(8):
                        nc.tensor.matmul(
                            c_ps[:, qc * 128:qc * 128 + 65],
                            pt_tiles[kc][:, qc * 128:(qc + 1) * 128],
                            d["vb"][kc][:, h * VST:h * VST + 65],
                            start=(kc == 0 and qc % 4 == 0),
                            stop=(kc == SCH - 1 and qc % 4 == 3),
                            skip_group_check=True,
                        )

                for kc in range(SCH):
                    s_ps = sps.tile([128, QW], F32, tag="s", name="s_ps")
                    for j in range(QW // 512):
                        nc.tensor.matmul(
                            s_ps[:, j * 512:(j + 1) * 512],
                            d["kt"][mi][pr:pr + 64, kc * 128:(kc + 1) * 128],
                            d["qt"][mi][pr:pr + 64,
                                        q0 + j * 512:q0 + (j + 1) * 512],
                            start=True, stop=True,
                        )
                    pt = ptp.tile([128, QW], BF16, tag="pt", name="pt")
                    if use_bias:
                        nc.scalar.activation(
                            pt[:], s_ps[:], EXP,
                            bias=mask_sb[:, kc:kc + 1], scale=0.125,
                        )
                    else:
                        nc.scalar.activation(pt[:], s_ps[:], EXP, scale=0.125)
                    pt_tiles[kc] = pt
                    for job in inserts.pop((r, u, kc), ()):
                        job()
                    if kc >= CTX_LAG:
                        ctx(kc - CTX_LAG)
                for kc in range(SCH - CTX_LAG, SCH):
                    ctx(kc)

                # normalize: out[q, d] = ctx[q, d] / ctx[q, 64] on Pool
                o_sb = osp.tile([128, 8 * DH], F32, tag="o")
                cs = csp.tile([128, 8 * 65], F32, tag="cs", name="cs")
                nc.vector.tensor_copy(
                    out=cs[:].rearrange("p (qc c) -> p qc c", c=65),
                    in_=c_ps[:].rearrange("p (qc c) -> p qc c", c=128)[:, :, 0:65])
                for qc in range(8):
                    nc.gpsimd.normalize_recip(
                        o_sb[:, qc * DH:(qc + 1) * DH],
                        cs[:, qc * 65:qc * 65 + 64],
                        cs[:, qc * 65 + 64:qc * 65 + 65])
                nc.sync.dma_start(
                    out[h, q0:q0 + QW, :].rearrange("(qc p) d -> p qc d", p=128),
                    o_sb[:].rearrange("p (qc d) -> p qc d", d=DH))

    assert not inserts, f"unconsumed insert jobs: {list(inserts)}"
    nc.compile()
    _cached[("nc", reps, use_bias)] = nc
    return nc


def shard_inputs(hidden_states, attention_mask, Wq, bq, Wk, bk, Wv, bv):
    """Host-side layout prep (no FLOPs): slice + transpose per core."""
    hidden_states = np.asarray(hidden_states, dtype=np.float32)
    attention_mask = np.asarray(attention_mask, dtype=np.float32)
    Wq, Wk, Wv = (np.asarray(w, dtype=np.float32) for w in (Wq, Wk, Wv))
    bq, bk, bv = (np.asarray(b, dtype=np.float32) for b in (bq, bk, bv))
    in_maps = []
    for c in range(N_CORES):
        b_idx, g = divmod(c, 2)
        cols = slice(g * DL, (g + 1) * DL)
        in_maps.append({
            "xT": np.ascontiguousarray(hidden_states[b_idx].T),
            "wq": np.ascontiguousarray(Wq[:, cols]),
            "wk": np.ascontiguousarray(Wk[:, cols]),
            "wv": np.ascontiguousarray(Wv[:, cols]),
            "bq": np.ascontiguousarray(bq[cols].reshape(MCH, 128).T),
            "bk": np.ascontiguousarray(bk[cols].reshape(MCH, 128).T),
            "bv": np.ascontiguousarray(bv[cols].reshape(1, DL)),
            "mask": np.ascontiguousarray(
                attention_mask[b_idx, 0, 0].reshape(SCH, 128).T),
        })
    return in_maps


def assemble_output(results):
    """results: per-core dicts with 'out' [HL, S, DH] -> full [B, S, D]."""
    final = np.empty((B, S, D), dtype=np.float32)
    for b_idx in range(B):
        parts = [results[2 * b_idx + g]["out"] for g in range(2)]  # [6, S, 64]
        ctx = np.concatenate(parts, axis=0)                        # [12, S, 64]
        final[b_idx] = ctx.transpose(1, 0, 2).reshape(S, D)
    return final


def kernel(**inputs) -> np.ndarray:
    use_bias = bool(np.any(np.asarray(inputs["attention_mask"])))
    nc = build_program(use_bias=use_bias)
    in_maps = shard_inputs(**inputs)
    res = run_bass_kernel_spmd(nc, in_maps, core_ids=list(range(N_CORES)))
    return assemble_output(res.results)


# revision 24
# speedup vs baseline: 1.3953x; 1.3953x over previous
"""BertSelfAttention (B=4, S=2048, D=768, H=12) on 8 Trainium2 NeuronCores.

Sharding: 8 cores = 4 batches x 2 head-groups (6 heads each). Per core,
for its (batch b, head-group g):

  Q^T = Wq_g^T @ x_b^T        [384, 2048] f32r  (d_local on partitions)
  K^T = Wk_g^T @ x_b^T        [384, 2048] f32r
  V   = x_b @ Wv_g            [2048, 384] -> bf16, ones-augmented per head
  per (head h, q-half):  flash-style kc pipeline
    S^T[k, q]  = sum_dh K^T[dh, k] Q^T[dh, q]        (PE f32r, psum [128,1024])
    P^T[k, q]  = exp(S^T/8 + mask[k])                (Act, fused -> bf16 SBUF)
    ctx[q, d|den] += P^T[k, q-chunk]^stat @ Vaug[k]  (PE bf16, q-major out,
                                                      ones col gives denom)
    out[q, d]  = ctx[q, 0:64] / ctx[q, 64]           (GpSimd normalize_recip
                                                      on the Pool engine)

The kernel is Act(exp)-bound: ~25.2M exp elements/core at 0.833 ns/elem,
~16.6 us of exp per (head, q-half) unit. Everything else hides in the
units' PE slack: scores run CTX_LAG kc ahead of ctx, the current rep's
mi=1/mi=2 projection chunks fill units 0-7, and the NEXT rep's input DMAs,
V projection, and mi=0 chunks run in units 6-11 (VB and the mi=0 Q^T/K^T
tiles are double-buffered across reps), so the steady-state per-rep period
approaches the pure-exp window.
"""

import numpy as np

import concourse.mybir as mybir
import concourse.tile as tile
from concourse import bacc
from concourse.bass_utils import run_bass_kernel_spmd

F32 = mybir.dt.float32
F32R = mybir.dt.float32r
BF16 = mybir.dt.bfloat16
U16 = mybir.dt.uint16
U32 = mybir.dt.uint32
ONE_BF16_BITS = 0x3F80

N_CORES = 8
B, S, D, H = 4, 2048, 768, 12
HL = 6           # heads per core
DH = 64          # head dim
DL = HL * DH     # 384: local output dim
DCH = D // 128   # 6 contraction chunks
MCH = DL // 128  # 3 partition chunks of Q^T/K^T (2 heads each)
SCH = S // 128   # 16 key chunks
VST = 66         # per-head stride in the ones-augmented V tile (64 d + 1 one)
QW = 1024        # q-columns per unit (psum/exp slice width)
NU = 2           # q-halves per head
CTX_LAG = 2      # kc distance between scores and ctx emission

_cached = {}


def build_program_w1(reps=1, use_bias=True):
    """reps>1 repeats the whole computation in one NEFF - used only by
    test.py to amortize dispatch overhead when measuring HW exec time.

    use_bias=False omits the attention-mask bias port on the exp
    activation (worth ~0.4 us per exp instruction on HW); kernel()
    selects the bias build only when the mask is actually nonzero."""
    if ("nc", reps, use_bias) in _cached:
        return _cached[("nc", reps, use_bias)]
    nc = bacc.Bacc("TRN2", target_bir_lowering=False, debug=False, num_devices=1)
    xT = nc.dram_tensor("xT", [D, S], F32, kind="ExternalInput").ap()
    wq = nc.dram_tensor("wq", [D, DL], F32, kind="ExternalInput").ap()
    wk = nc.dram_tensor("wk", [D, DL], F32, kind="ExternalInput").ap()
    wv = nc.dram_tensor("wv", [D, DL], F32, kind="ExternalInput").ap()
    bq = nc.dram_tensor("bq", [128, MCH], F32, kind="ExternalInput").ap()
    bk = nc.dram_tensor("bk", [128, MCH], F32, kind="ExternalInput").ap()
    bv = nc.dram_tensor("bv", [1, DL], F32, kind="ExternalInput").ap()
    mask = nc.dram_tensor("mask", [128, SCH], F32, kind="ExternalInput").ap()
    out = nc.dram_tensor("out", [HL, S, DH], F32, kind="ExternalOutput").ap()

    EXP = mybir.ActivationFunctionType.Exp

    with tile.TileContext(nc) as tc, \
         tc.tile_pool(name="persist", bufs=1) as persist, \
         tc.tile_pool(name="load", bufs=1) as load, \
         tc.tile_pool(name="pj", bufs=2, space="PSUM") as pj, \
         tc.tile_pool(name="sps", bufs=2, space="PSUM") as sps, \
         tc.tile_pool(name="cps", bufs=1, space="PSUM") as cps, \
         tc.tile_pool(name="ptp", bufs=CTX_LAG + 3) as ptp, \
         tc.tile_pool(name="csp", bufs=2) as csp, \
         tc.tile_pool(name="osp", bufs=2) as osp:

        # ---- per-rep resource handles (double-buffered across reps where
        # the next rep's projections overlap this rep's attention) ----
        R = []
        for r in range(reps):
            par = r % 2 if reps > 1 else 0
            d = {}
            d["qt"] = [persist.tile([128, S], F32R, tag=f"qt{i}{par if i == 0 else 0}",
                                    name=f"qt{i}_r{r}") for i in range(MCH)]
            d["kt"] = [persist.tile([128, S], F32R, tag=f"kt{i}{par if i == 0 else 0}",
                                    name=f"kt{i}_r{r}") for i in range(MCH)]
            d["vb"] = [persist.tile([128, HL * VST], BF16, tag=f"vb{i}{par}",
                                    name=f"vb{i}_r{r}") for i in range(SCH)]
            d["xt"] = [load.tile([128, S], F32R, tag=f"xt{dc}", name=f"xt{dc}_r{r}")
                       for dc in range(DCH)]
            d["w"] = {nm: [load.tile([128, DL], F32R, tag=f"w{nm}{dc}",
                                     name=f"w{nm}{dc}_r{r}") for dc in range(DCH)]
                      for nm in ("q", "k", "v")}
            d["bq"] = load.tile([128, MCH], F32, tag="bq", name=f"bq_r{r}")
            d["bk"] = load.tile([128, MCH], F32, tag="bk", name=f"bk_r{r}")
            d["bv"] = load.tile([1, DL], F32, tag="bv", name=f"bv_r{r}")
            R.append(d)

        mask_sb = persist.tile([128, SCH], F32, tag="mask")
        nc.sync.dma_start(mask_sb[:], mask[:])
        for par in range(min(reps, 2)):
            for i in range(SCH):
                # ones column (col 64 of each head block): softmax denominator
                nc.vector.memset(
                    R[par]["vb"][i][:].bitcast(U16).rearrange(
                        "p (h j) -> p h j", j=VST)[:, :, 64:65],
                    ONE_BF16_BITS)
        ones_row = persist.tile([1, 128], F32, tag="ones_row")
        nc.vector.memset(ones_row[:], 1.0)

        def input_dmas(r):
            d = R[r]
            for dc in range(DCH):
                nc.sync.dma_start(d["w"]["q"][dc][:],
                                  wq[dc * 128:(dc + 1) * 128, :].bitcast(F32R))
            nc.sync.dma_start(d["bq"][:], bq[:])
            nc.sync.dma_start(d["bk"][:], bk[:])
            for dc in range(DCH):
                nc.sync.dma_start(d["xt"][dc][:],
                                  xT[dc * 128:(dc + 1) * 128, :].bitcast(F32R))
            for dc in range(DCH):
                nc.sync.dma_start(d["w"]["k"][dc][:],
                                  wk[dc * 128:(dc + 1) * 128, :].bitcast(F32R))
            for dc in range(DCH):
                nc.sync.dma_start(d["w"]["v"][dc][:],
                                  wv[dc * 128:(dc + 1) * 128, :].bitcast(F32R))
            nc.sync.dma_start(d["bv"][:], bv[:])

        def qk_chunk(r, wn, mi, q):
            """One [128, 512] column chunk of rep r's Q^T or K^T."""
            d = R[r]
            bt, dst = (d["bq"], d["qt"]) if wn == "q" else (d["bk"], d["kt"])
            ps = pj.tile([128, 512], F32, tag="qk", name="ps_qk")
            for dc in range(DCH):
                nc.tensor.matmul(
                    ps[:],
                    d["w"][wn][dc][:, mi * 128:(mi + 1) * 128],
                    d["xt"][dc][:, q * 512:(q + 1) * 512],
                    start=(dc == 0), stop=(dc == DCH - 1),
                )
            nc.vector.tensor_scalar_add(
                dst[mi][:, q * 512:(q + 1) * 512], ps[:], bt[:, mi:mi + 1])

        def v_chunk(r, sc):
            """One [128 seq, 384] chunk of rep r's V -> bf16 augmented VB."""
            d = R[r]
            pst = pj.tile([128, 512], F32, tag="qk", name="ps_v")
            ps = pst[:, 0:DL]
            for dc in range(DCH):
                nc.tensor.matmul(
                    ps,
                    d["xt"][dc][:, sc * 128:(sc + 1) * 128],
                    d["w"]["v"][dc][:],
                    start=(dc == 0), stop=False,
                )
            nc.tensor.matmul(ps, ones_row[:], d["bv"][:],
                             start=False, stop=True)
            nc.vector.tensor_copy(
                out=d["vb"][sc][:].rearrange("p (h j) -> p h j", j=VST)[:, :, 0:64],
                in_=ps.rearrange("p (h j) -> p h j", j=64),
            )

        # ---- insert schedule: jobs keyed by (rep, unit, kc) ----
        inserts = {}

        def add(r, u, kc, job):
            inserts.setdefault((r, u, kc), []).append(job)

        for r in range(reps):
            # this rep's mi=1 chunks in units 1-3 (deadline: unit 4 = h2)
            for j in range(8):
                u, kc = 1 + j // 3, (j % 3) * 5 + 1
                add(r, u, kc, lambda r=r, wn=("q", "k")[j % 2], q=j // 2:
                    qk_chunk(r, wn, 1, q))
            # this rep's mi=2 chunks in units 4-6 (deadline: unit 8 = h4;
            # all of this rep's xt reads end by unit 6 kc 8 so the next
            # rep's input DMAs can be emitted right after)
            for j in range(8):
                u, kc = 4 + j // 3, (j % 3) * 5 + 3
                add(r, u, kc, lambda r=r, wn=("q", "k")[j % 2], q=j // 2:
                    qk_chunk(r, wn, 2, q))
            if r == 0:
                # no previous rep: V and remaining mi=0 go into unit 0
                for sc in range(SCH):
                    add(0, 0, sc, lambda sc=sc: v_chunk(0, sc))
                add(0, 0, 1, lambda: qk_chunk(0, "k", 0, 1))
                add(0, 0, 5, lambda: qk_chunk(0, "k", 0, 2))
                add(0, 0, 9, lambda: qk_chunk(0, "k", 0, 3))
                add(0, 0, 11, lambda: qk_chunk(0, "q", 0, 2))
                add(0, 0, 13, lambda: qk_chunk(0, "q", 0, 3))
            if r + 1 < reps:
                # prefetch for the NEXT rep into this rep's tail units
                # (after this rep's last xt/w read at unit 6 kc 13)
                add(r, 6, 15, lambda r=r: input_dmas(r + 1))
                for sc in range(SCH):  # V: units 7-10, 4 chunks each
                    add(r, 7 + sc // 4, (sc % 4) * 4 + 2,
                        lambda r=r, sc=sc: v_chunk(r + 1, sc))
                for j in range(8):     # mi=0: units 9-11, 3/3/2 chunks
                    u, kc = 9 + j // 3, (j % 3) * 5 + 3
                    add(r, u, kc, lambda r=r, wn=("q", "k")[j % 2], q=j // 2:
                        qk_chunk(r + 1, wn, 0, q))

        # ---- rep 0 prologue ----
        input_dmas(0)
        # PE warm-up during the input-DMA window: junk matmuls ramp the
        # p-state so the first real chunks run at full speed.
        warm_in = persist.tile([1, 512], F32R, tag="warm")
        nc.vector.memset(warm_in[:].bitcast(U32), 0)
        warm_ps = pj.tile([128, 512], F32, tag="qk", name="warm_ps")
        for _ in range(14):
            nc.tensor.matmul(warm_ps[:], ones_row[:].bitcast(F32R),
                             warm_in[:], start=True, stop=True)
        # just enough of mi=0 for unit 0 to start (QT cols 0:1024 for the
        # moving operand, KT cols 0:512 covering kc 0-3)
        qk_chunk(0, "q", 0, 0)
        qk_chunk(0, "q", 0, 1)
        qk_chunk(0, "k", 0, 0)

        # ---- attention units ----
        for r in range(reps):
            d = R[r]
            for u in range(HL * NU):
                h, half = divmod(u, NU)
                mi, pr = h // 2, (h % 2) * 64
                q0 = half * QW
                c_ps = cps.tile([128, 1024], F32, tag="c", name="c_ps")
                pt_tiles = [None] * SCH

                def ctx(kc):
                    # PSUM accumulation groups are bank-granular (2 KB zero
                    # regions): start only on the first matmul touching each
                    # bank (pending-zero lazily zeroes the other slots on
                    # their first write), stop only on the last.
                    for qc in range(8):
                        nc.tensor.matmul(
                            c_ps[:, qc * 128:qc * 128 + 65],
                            pt_tiles[kc][:, qc * 128:(qc + 1) * 128],
                            d["vb"][kc][:, h * VST:h * VST + 65],
                            start=(kc == 0 and qc % 4 == 0),
                            stop=(kc == SCH - 1 and qc % 4 == 3),
                            skip_group_check=True,
                        )

                for kc in range(SCH):
                    s_ps = sps.tile([128, QW], F32, tag="s", name="s_ps")
                    for j in range(QW // 512):
                        nc.tensor.matmul(
                            s_ps[:, j * 512:(j + 1) * 512],
                            d["kt"][mi][pr:pr + 64, kc * 128:(kc + 1) * 128],
                            d["qt"][mi][pr:pr + 64,
                                        q0 + j * 512:q0 + (j + 1) * 512],
                            start=True, stop=True,
                        )
                    pt = ptp.tile([128, QW], BF16, tag="pt", name="pt")
                    if use_bias:
                        nc.scalar.activation(
                            pt[:], s_ps[:], EXP,
                            bias=mask_sb[:, kc:kc + 1], scale=0.125,
                        )
                    else:
                        nc.scalar.activation(pt[:], s_ps[:], EXP, scale=0.125)
                    pt_tiles[kc] = pt
                    for job in inserts.pop((r, u, kc), ()):
                        job()
                    if kc >= CTX_LAG:
                        ctx(kc - CTX_LAG)
                for kc in range(SCH - CTX_LAG, SCH):
                    ctx(kc)

                # normalize: out[q, d] = ctx[q, d] / ctx[q, 64] on Pool
                o_sb = osp.tile([128, 8 * DH], F32, tag="o")
                cs = csp.tile([128, 8 * 65], F32, tag="cs", name="cs")
                nc.vector.tensor_copy(
                    out=cs[:].rearrange("p (qc c) -> p qc c", c=65),
                    in_=c_ps[:].rearrange("p (qc c) -> p qc c", c=128)[:, :, 0:65])
                for qc in range(8):
                    nc.gpsimd.normalize_recip(
                        o_sb[:, qc * DH:(qc + 1) * DH],
                        cs[:, qc * 65:qc * 65 + 64],
                        cs[:, qc * 65 + 64:qc * 65 + 65])
                nc.sync.dma_start(
                    out[h, q0:q0 + QW, :].rearrange("(qc p) d -> p qc d", p=128),
                    o_sb[:].rearrange("p (qc d) -> p qc d", d=DH))

    assert not inserts, f"unconsumed insert jobs: {list(inserts)}"
    nc.compile()
    _cached[("nc", reps, use_bias)] = nc
    return nc



SLOTCOL = [(j // 7) * 512 + (j % 7) * 65 for j in range(16)]  # 7+7+2 per bank


def build_program_w2(reps=1, use_bias=True, probe=None):
    """Variant: 2048-wide exp slices (one unit per head). Scores psum is
    single-buffered [128, 2048]; ctx uses a [128, 1536] psum with 16
    65-col slots packed 7+7+2 per bank (pending-zero handles sub-bank
    group starts)."""
    if ("w2", reps, use_bias, probe) in _cached:
        return _cached[("w2", reps, use_bias, probe)]
    nc = bacc.Bacc("TRN2", target_bir_lowering=False, debug=False, num_devices=1)
    xT = nc.dram_tensor("xT", [D, S], F32, kind="ExternalInput").ap()
    wq = nc.dram_tensor("wq", [D, DL], F32, kind="ExternalInput").ap()
    wk = nc.dram_tensor("wk", [D, DL], F32, kind="ExternalInput").ap()
    wv = nc.dram_tensor("wv", [D, DL], F32, kind="ExternalInput").ap()
    bq = nc.dram_tensor("bq", [128, MCH], F32, kind="ExternalInput").ap()
    bk = nc.dram_tensor("bk", [128, MCH], F32, kind="ExternalInput").ap()
    bv = nc.dram_tensor("bv", [1, DL], F32, kind="ExternalInput").ap()
    mask = nc.dram_tensor("mask", [128, SCH], F32, kind="ExternalInput").ap()
    out = nc.dram_tensor("out", [HL, S, DH], F32, kind="ExternalOutput").ap()

    EXP = mybir.ActivationFunctionType.Exp

    with tile.TileContext(nc) as tc, \
         tc.tile_pool(name="persist", bufs=1) as persist, \
         tc.tile_pool(name="load", bufs=1) as load, \
         tc.tile_pool(name="pj", bufs=1, space="PSUM") as pj, \
         tc.tile_pool(name="sps", bufs=1, space="PSUM") as sps, \
         tc.tile_pool(name="cps", bufs=1, space="PSUM") as cps, \
         tc.tile_pool(name="ptp", bufs=4) as ptp, \
         tc.tile_pool(name="csp", bufs=2) as csp, \
         tc.tile_pool(name="osp", bufs=2) as osp:

        R = []
        for r in range(reps):
            par = r % 2 if reps > 1 else 0
            d = {}
            d["qt"] = [persist.tile([128, S], F32R, tag=f"qt{i}{par if i == 0 else 0}",
                                    name=f"qt{i}_r{r}") for i in range(MCH)]
            d["kt"] = [persist.tile([128, S], F32R, tag=f"kt{i}{par if i == 0 else 0}",
                                    name=f"kt{i}_r{r}") for i in range(MCH)]
            d["vb"] = [persist.tile([128, HL * VST], BF16, tag=f"vb{i}{par}",
                                    name=f"vb{i}_r{r}") for i in range(SCH)]
            d["xt"] = [load.tile([128, S], F32R, tag=f"xt{dc}", name=f"xt{dc}_r{r}")
                       for dc in range(DCH)]
            d["w"] = {nm: [load.tile([128, DL], F32R, tag=f"w{nm}{dc}",
                                     name=f"w{nm}{dc}_r{r}") for dc in range(DCH)]
                      for nm in ("q", "k", "v")}
            d["bq"] = load.tile([128, MCH], F32, tag="bq", name=f"bq_r{r}")
            d["bk"] = load.tile([128, MCH], F32, tag="bk", name=f"bk_r{r}")
            d["bv"] = load.tile([1, DL], F32, tag="bv", name=f"bv_r{r}")
            R.append(d)

        mask_sb = persist.tile([128, SCH], F32, tag="mask")
        nc.sync.dma_start(mask_sb[:], mask[:])
        for par in range(min(reps, 2)):
            for i in range(SCH):
                nc.vector.memset(
                    R[par]["vb"][i][:].bitcast(U16).rearrange(
                        "p (h j) -> p h j", j=VST)[:, :, 64:65],
                    ONE_BF16_BITS)
        ones_row = persist.tile([1, 128], F32, tag="ones_row")
        nc.vector.memset(ones_row[:], 1.0)

        def input_dmas(r):
            d = R[r]
            for dc in range(DCH):
                nc.sync.dma_start(d["w"]["q"][dc][:],
                                  wq[dc * 128:(dc + 1) * 128, :].bitcast(F32R))
            nc.sync.dma_start(d["bq"][:], bq[:])
            nc.sync.dma_start(d["bk"][:], bk[:])
            for dc in range(DCH):
                nc.sync.dma_start(d["xt"][dc][:],
                                  xT[dc * 128:(dc + 1) * 128, :].bitcast(F32R))
            for dc in range(DCH):
                nc.sync.dma_start(d["w"]["k"][dc][:],
                                  wk[dc * 128:(dc + 1) * 128, :].bitcast(F32R))
            for dc in range(DCH):
                nc.sync.dma_start(d["w"]["v"][dc][:],
                                  wv[dc * 128:(dc + 1) * 128, :].bitcast(F32R))
            nc.sync.dma_start(d["bv"][:], bv[:])

        def qk_chunk(r, wn, mi, q):
            d = R[r]
            bt, dst = (d["bq"], d["qt"]) if wn == "q" else (d["bk"], d["kt"])
            ps = pj.tile([128, 512], F32, tag="qk", name="ps_qk")
            for dc in range(DCH):
                nc.tensor.matmul(
                    ps[:],
                    d["w"][wn][dc][:, mi * 128:(mi + 1) * 128],
                    d["xt"][dc][:, q * 512:(q + 1) * 512],
                    start=(dc == 0), stop=(dc == DCH - 1),
                )
            nc.vector.tensor_scalar_add(
                dst[mi][:, q * 512:(q + 1) * 512], ps[:], bt[:, mi:mi + 1])

        def v_chunk(r, sc):
            d = R[r]
            pst = pj.tile([128, 512], F32, tag="qk", name="ps_v")
            ps = pst[:, 0:DL]
            for dc in range(DCH):
                nc.tensor.matmul(
                    ps,
                    d["xt"][dc][:, sc * 128:(sc + 1) * 128],
                    d["w"]["v"][dc][:],
                    start=(dc == 0), stop=False,
                )
            nc.tensor.matmul(ps, ones_row[:], d["bv"][:],
                             start=False, stop=True)
            nc.vector.tensor_copy(
                out=d["vb"][sc][:].rearrange("p (h j) -> p h j", j=VST)[:, :, 0:64],
                in_=ps.rearrange("p (h j) -> p h j", j=64),
            )

        inserts = {}

        def add(r, u, kc, job):
            inserts.setdefault((r, u, kc), []).append(job)

        for r in range(reps):
            for j in range(8):   # mi=1 in units 0-1 (deadline unit 2)
                add(r, j // 4, (j % 4) * 4 + 3,
                    lambda r=r, wn=("q", "k")[j % 2], q=j // 2:
                    qk_chunk(r, wn, 1, q))
            for j in range(8):   # mi=2 in units 2-3 (deadline unit 4)
                add(r, 2 + j // 4, (j % 4) * 4 + 3,
                    lambda r=r, wn=("q", "k")[j % 2], q=j // 2:
                    qk_chunk(r, wn, 2, q))
            if r == 0:
                for sc in range(SCH):
                    add(0, 0, sc, lambda sc=sc: v_chunk(0, sc))
                add(0, 0, 1, lambda: qk_chunk(0, "k", 0, 1))
                add(0, 0, 5, lambda: qk_chunk(0, "k", 0, 2))
                add(0, 0, 9, lambda: qk_chunk(0, "k", 0, 3))
            if r + 1 < reps:
                add(r, 4, 1, lambda r=r: input_dmas(r + 1))
                for sc in range(SCH):  # V: units 4-5, odd kc
                    add(r, 4 + sc // 8, (sc % 8) * 2 + 1,
                        lambda r=r, sc=sc: v_chunk(r + 1, sc))
                for j in range(8):     # mi=0: unit 5, even kc
                    add(r, 5, j * 2,
                        lambda r=r, wn=("q", "k")[j % 2], q=j // 2:
                        qk_chunk(r + 1, wn, 0, q))

        input_dmas(0)
        warm_in = persist.tile([1, 512], F32R, tag="warm")
        nc.vector.memset(warm_in[:].bitcast(U32), 0)
        warm_ps = pj.tile([128, 512], F32, tag="qk", name="warm_ps")
        for _ in range(14):
            nc.tensor.matmul(warm_ps[:], ones_row[:].bitcast(F32R),
                             warm_in[:], start=True, stop=True)
        for q in range(4):
            qk_chunk(0, "q", 0, q)
        qk_chunk(0, "k", 0, 0)

        for r in range(reps):
            d = R[r]
            for u in range(HL):
                h = u
                mi, pr = h // 2, (h % 2) * 64
                c_ps = cps.tile([128, 1536], F32, tag="c", name="c_ps")
                pt_tiles = [None] * SCH

                def ctx(kc):
                    nqc = 1 if probe == "thinctx" else 16
                    for qc in range(nqc):
                        col = SLOTCOL[qc]
                        nc.tensor.matmul(
                            c_ps[:, col:col + 65],
                            pt_tiles[kc][:, qc * 128:(qc + 1) * 128],
                            d["vb"][kc][:, h * VST:h * VST + 65],
                            start=(kc == 0 and qc % 7 == 0),
                            stop=(kc == SCH - 1 and (qc % 7 == 6 or qc == 15)),
                            skip_group_check=True,
                        )

                for kc in range(SCH):
                    s_ps = sps.tile([128, S], F32, tag="s", name="s_ps")
                    for j in range(4):
                        nc.tensor.matmul(
                            s_ps[:, j * 512:(j + 1) * 512],
                            d["kt"][mi][pr:pr + 64, kc * 128:(kc + 1) * 128],
                            d["qt"][mi][pr:pr + 64, j * 512:(j + 1) * 512],
                            start=True, stop=True,
                        )
                    pt = ptp.tile([128, S], BF16, tag="pt", name="pt")
                    func = (mybir.ActivationFunctionType.Copy
                            if probe == "copyact" else EXP)
                    if probe == "copyact":
                        nc.scalar.activation(pt[:], s_ps[:], func, scale=0.125)
                    elif use_bias:
                        nc.scalar.activation(
                            pt[:], s_ps[:], func,
                            bias=mask_sb[:, kc:kc + 1], scale=0.125,
                        )
                    else:
                        nc.scalar.activation(pt[:], s_ps[:], func, scale=0.125)
                    pt_tiles[kc] = pt
                    for job in inserts.pop((r, u, kc), ()):
                        job()
                    if kc >= 2:
                        ctx(kc - 2)
                for kc in range(SCH - 2, SCH):
                    ctx(kc)

                o_sb = osp.tile([128, 16 * DH], F32, tag="o")
                cs = csp.tile([128, 16 * 65], F32, tag="cs", name="cs")
                for b, nsl in ((0, 7), (1, 7), (2, 2)):
                    nc.vector.tensor_copy(
                        out=cs[:, b * 7 * 65:(b * 7 + nsl) * 65].rearrange(
                            "p (j c) -> p j c", c=65),
                        in_=c_ps[:, b * 512:b * 512 + nsl * 65].rearrange(
                            "p (j c) -> p j c", c=65))
                for qc in range(16):
                    nc.gpsimd.normalize_recip(
                        o_sb[:, qc * DH:(qc + 1) * DH],
                        cs[:, qc * 65:qc * 65 + 64],
                        cs[:, qc * 65 + 64:qc * 65 + 65])
                nc.sync.dma_start(
                    out[h].rearrange("(qc p) d -> p qc d", p=128),
                    o_sb[:].rearrange("p (qc d) -> p qc d", d=DH))

    assert not inserts, f"unconsumed insert jobs: {list(inserts)}"
    nc.compile()
    _cached[("w2", reps, use_bias, probe)] = nc
    return nc


build_program = build_program_w2  # current best on HW


def shard_inputs(hidden_states, attention_mask, Wq, bq, Wk, bk, Wv, bv):
    """Host-side layout prep (no FLOPs): slice + transpose per core."""
    hidden_states = np.asarray(hidden_states, dtype=np.float32)
    attention_mask = np.asarray(attention_mask, dtype=np.float32)
    Wq, Wk, Wv = (np.asarray(w, dtype=np.float32) for w in (Wq, Wk, Wv))
    bq, bk, bv = (np.asarray(b, dtype=np.float32) for b in (bq, bk, bv))
    in_maps = []
    for c in range(N_CORES):
        b_idx, g = divmod(c, 2)
        cols = slice(g * DL, (g + 1) * DL)
        in_maps.append({
            "xT": np.ascontiguousarray(hidden_states[b_idx].T),
            "wq": np.ascontiguousarray(Wq[:, cols]),
            "wk": np.ascontiguousarray(Wk[:, cols]),
            "wv": np.ascontiguousarray(Wv[:, cols]),
            "bq": np.ascontiguousarray(bq[cols].reshape(MCH, 128).T),
            "bk": np.ascontiguousarray(bk[cols].reshape(MCH, 128).T),
            "bv": np.ascontiguousarray(bv[cols].reshape(1, DL)),
            "mask": np.ascontiguousarray(
                attention_mask[b_idx, 0, 0].reshape(SCH, 128).T),
        })
    return in_maps


def assemble_output(results):
    """results: per-core dicts with 'out' [HL, S, DH] -> full [B, S, D]."""
    final = np.empty((B, S, D), dtype=np.float32)
    for b_idx in range(B):
        parts = [results[2 * b_idx + g]["out"] for g in range(2)]  # [6, S, 64]
        ctx = np.concatenate(parts, axis=0)                        # [12, S, 64]
        final[b_idx] = ctx.transpose(1, 0, 2).reshape(S, D)
    return final


def kernel(**inputs) -> np.ndarray:
    nc = build_program()
    in_maps = shard_inputs(**inputs)
    res = run_bass_kernel_spmd(nc, in_maps, core_ids=list(range(N_CORES)))
    return assemble_output(res.results)
